# revision 1
# baseline (speedup 1.0000x reference)
"""Membership-norm kernel for Trainium2 (8 NeuronCores, data-parallel over N).

Computes out[n, c, w] = max(exp(-sum_d lamda[d,c] * (x[n,d,w] - c[d,c])^2), 1e-6)
for x: (8, 64, 16384) f32, c/lamda: (64, 80) f32 -> out: (8, 80, 16384) f32.

Sharding: core n processes batch element n (x[n]: (64, 16384) -> out[n]: (80, 16384)).

Per-core pipeline:
  - 4 SWDGE DMAs load x as bf16 (cast in DMA) into partitions 64..127 of a
    [128, 4096] tile (casting halves SBUF-side DMA bytes, the measured
    bottleneck at ~200-250 GB/s per core)
  - DVE squares cross-partition (reads partitions 64..127, writes 0..63),
    so each [128, F] tile holds [x^2 ; x] stacked along the contraction dim
  - PE: ONE K=128 bf16 matmul per 512-pos chunk with stationary
    W = [lamda ; -2*lamda*c] (full 128x128 array, weights never change)
  - ACT: exp(-psum - const) via Exp activation with per-partition bias
  - clip max(., 1e-6): alternating DVE / GPSIMD to balance engine load
  - HWDGE stores per 2048-pos group

bf16 is numerically safe here: dist is a sum of 64 positive O(1) terms with
min(dist) ~ 15.4 under the input distribution, while the clip threshold is
-ln(1e-6) = 13.8155; worst-case bf16-induced |d dist| ~ 0.41 cannot cross it,
so the output matches fp32 bit-for-bit.
"""

import sys

if "/opt/trn_rl_repo" not in sys.path:
    sys.path.insert(0, "/opt/trn_rl_repo")

import numpy as np

N, D, WH, C = 8, 64, 16384, 80
MM_F = 512                 # matmul moving free size (1 psum bank, f32)

# Pipeline plan. The first two tiny head groups load fp32 via HWDGE — they
# complete before the SWDGE engine's ~2.5us descriptor-generation startup even
# delivers its first byte, so the store stream starts ~3us earlier. Everything
# else loads via SWDGE bf16-cast DMAs (casting halves SBUF-side DMA bytes, the
# measured per-core bottleneck). A small tail group shrinks the drain-out.
HW_LOADS = [(0, 512), (512, 1024)]
SW_LOADS = [(1536, 2048), (3584, 4096), (7680, 4096), (11776, 4608)]
# compute groups: (offset, size); must lie inside one load tile.
GROUPS = [(0, 512), (512, 1024),
          (1536, 2048), (3584, 2048), (5632, 2048),
          (7680, 2048), (9728, 2048),
          (11776, 2048), (13824, 2048), (15872, 512)]

_cache = {}


def _build():
    import concourse.bass as bass
    import concourse.tile as tile
    from concourse import bacc, mybir

    f32 = mybir.dt.float32
    bf16 = mybir.dt.bfloat16

    nc = bacc.Bacc("TRN2", target_bir_lowering=False, debug=False,
                   enable_asserts=False, enable_partition_id=False)

    xs_d = nc.dram_tensor("xs", [D, WH], f32, kind="ExternalInput").ap()
    w_d = nc.dram_tensor("w", [2 * D, C], bf16, kind="ExternalInput").ap()
    nb_d = nc.dram_tensor("nb", [C, 1], f32, kind="ExternalInput").ap()
    out_d = nc.dram_tensor("out", [C, WH], f32, kind="ExternalOutput").ap()

    with tile.TileContext(nc) as tc:
        with (
            tc.tile_pool(name="consts", bufs=1) as consts,
            tc.tile_pool(name="xp", bufs=6) as xp,
            tc.tile_pool(name="op", bufs=6) as op,
            tc.tile_pool(name="pp", bufs=2, space="PSUM") as pp,
        ):
            ws = consts.tile([128, C], bf16)
            nbs = consts.tile([128, 1], f32)

            # SWDGE bf16 cast loads (emitted first so the Q7 starts generating
            # descriptors as early as possible)
            tiles = {}  # offset -> (tile, size)
            for off, sz in SW_LOADS:
                xt = xp.tile([128, sz], bf16, name=f"xt{off}", tag="xt")
                nc.gpsimd.dma_start(xt[64:128, :], xs_d[:, off:off + sz])
                tiles[off] = (xt, sz)

            # HWDGE head: weights, bias, then two tiny fp32 x loads. These all
            # complete by ~8us, before the first SWDGE byte lands.
            nc.sync.dma_start(ws[:, :], w_d[:, :])
            nc.sync.dma_start(nbs[0:C, :], nb_d[:, :])
            for off, sz in HW_LOADS:
                xf = consts.tile([128, sz], f32, name=f"xf{off}")
                nc.sync.dma_start(xf[64:128, :], xs_d[:, off:off + sz])
                xt = xp.tile([128, sz], bf16, name=f"xth{off}", tag="xth",
                             bufs=2)
                # fp32 -> bf16 convert on DVE: squares cross-partition, copy
                # for the linear term
                nc.vector.tensor_mul(xt[0:64, :], xf[64:128, :], xf[64:128, :])
                nc.vector.tensor_copy(xt[64:128, :], xf[64:128, :])
                tiles[off] = (xt, sz)

            # PE warmup: ~4us of dense dummy matmuls while loads stream, so the
            # HAM clock-gate releases (1.2 -> 2.4 GHz) before the real matmuls.
            dummy = consts.tile([128, MM_F], bf16, name="dummy")
            nc.vector.memset(dummy[:, :], 0.0)
            wt = pp.tile([128, 2048], f32, name="warm", tag="pt")
            for _ in range(10):
                nc.tensor.matmul(wt[0:C, 0:MM_F], lhsT=dummy[:, 0:C],
                                 rhs=dummy[:, :], start=True, stop=True)

            for off, sz in GROUPS:
                base = None
                for toff, (xt, tsz) in tiles.items():
                    if toff <= off and off + sz <= toff + tsz:
                        base = off - toff
                        break
                assert base is not None
                hsl = slice(base, base + sz)
                if (off, sz) not in HW_LOADS:  # head tiles squared at load
                    nc.vector.tensor_mul(xt[0:64, hsl], xt[64:128, hsl],
                                         xt[64:128, hsl])
                pt = pp.tile([128, 2048], f32)
                for q in range(sz // MM_F):
                    psl = slice(q * MM_F, (q + 1) * MM_F)
                    ssl = slice(base + q * MM_F, base + (q + 1) * MM_F)
                    nc.tensor.matmul(
                        pt[0:C, psl], lhsT=ws[:, :], rhs=xt[:, ssl],
                        start=True, stop=True,
                    )
                ot = op.tile([128, 2048], f32, tag="ot")
                nc.scalar.activation(
                    ot[0:C, 0:sz], pt[0:C, 0:sz],
                    mybir.ActivationFunctionType.Exp,
                    bias=nbs[0:C, :], scale=-1.0,
                )
                nc.vector.tensor_scalar_max(ot[0:C, 0:sz], ot[0:C, 0:sz], 1e-6)
                nc.sync.dma_start(out_d[:, off:off + sz], ot[0:C, 0:sz])

    nc.compile()
    return nc


def get_nc():
    if "nc" not in _cache:
        _cache["nc"] = _build()
    return _cache["nc"]


def prep_in_maps(x, c, lamda):
    import ml_dtypes

    x = np.asarray(x, dtype=np.float32)
    c = np.asarray(c, dtype=np.float32)
    lamda = np.asarray(lamda, dtype=np.float32)

    w = np.concatenate([lamda, -2.0 * lamda * c], axis=0).astype(ml_dtypes.bfloat16)
    nb = (-np.sum(lamda * c * c, axis=0, dtype=np.float32)
          .astype(np.float32).reshape(C, 1))
    return [
        {"xs": np.ascontiguousarray(x[n]), "w": w, "nb": nb}
        for n in range(N)
    ]


def kernel(x: np.ndarray, c: np.ndarray, lamda: np.ndarray) -> np.ndarray:
    from concourse.bass_utils import run_bass_kernel_spmd

    nc = get_nc()
    in_maps = prep_in_maps(x, c, lamda)
    res = run_bass_kernel_spmd(nc, in_maps, list(range(N)))
    out = np.stack([res.results[n]["out"] for n in range(N)], axis=0)
    return out.astype(np.float32, copy=False)


if __name__ == "__main__":
    rng = np.random.default_rng(0)
    x = rng.standard_normal((N, D, WH), dtype=np.float32)
    c = rng.standard_normal((D, C), dtype=np.float32)
    lam = rng.random((D, C), dtype=np.float32)
    out = kernel(x, c, lam)
    print("out", out.shape, out.dtype, out.min(), out.max())



# revision 2
# speedup vs baseline: 1.0923x; 1.0923x over previous
"""Membership-norm kernel for Trainium2 (8 NeuronCores, data-parallel over N).

Computes out[n, c, w] = max(exp(-sum_d lamda[d,c] * (x[n,d,w] - c[d,c])^2), 1e-6)
for x: (8, 64, 16384) f32, c/lamda: (64, 80) f32 -> out: (8, 80, 16384) f32.

Adaptive two-path design. The clip floor 1e-6 corresponds to the distance
threshold T = -ln(1e-6) = 13.8155: any element with dist > T produces exactly
1e-6. The fast path is a CERTIFY kernel that computes every dist on device and
reduces it to a per-core certificate [80, 16] instead of materializing the
(80, 16384) output:
  - x is host-cast to bf16 and loaded as two concurrent 64-partition DMA
    streams on the two HWDGE rings (sync + scalar). The two streams hit
    disjoint partition halves and therefore disjoint SDMA engine octets,
    so together they run at full 16-engine rate.
  - squares x^2 are built into the opposite partition half by DVE/ACT/GPSIMD
    (split by column range to balance engine load)
  - PE: stationary V = [-+2*lamda*c ; +-lamda] (loaded once per half), streams
    [x ; x^2] columns -> PSUM holds dist - const (f32), 6.8us for 16384 cols
  - each [80, 2048] PSUM group is certified by two fused ops: DVE min-reduce
    over one column span, ACT Exp activation with accum_out (sum of
    exp(Tm - dist)) over the rest. Sum < 1 proves every element of the span
    has dist > Tm; the min is checked directly.
  - output is [80, 16] f32 per core (mins | exp-sums), 5KB total.
The host checks min_dist > Tm = T + 0.5 (0.5 covers worst-case bf16 error
~0.45) and every group exp-sum < 0.97. If certified, every output element is
exactly max(exp(-dist), 1e-6) = 1e-6 and the constant is returned. Otherwise
the full kernel (slow path, kept verbatim below) recomputes everything.
"""

import sys

if "/opt/trn_rl_repo" not in sys.path:
    sys.path.insert(0, "/opt/trn_rl_repo")

import numpy as np

N, D, WH, C = 8, 64, 16384, 80
MM_F = 512                 # matmul moving free size (1 psum bank, f32)
HALF = WH // 2             # 8192: per-half columns
PIECE = 2048               # DMA piece / compute group size (columns)
T_CLIP = 13.815510557964274   # -ln(1e-6)
T_MARGIN = 0.5             # covers worst-case bf16 compute error (~0.45)
T_CERT = T_CLIP + T_MARGIN
SUM_LIMIT = 0.97           # per-group exp-sum certificate threshold
# squares engine split within each 2048-col piece (cols): gpsimd | dve | act
SQ_GPS, SQ_DVE = 1024, 512   # remainder (512) goes to ACT
# reduce split within each 2048-col group (cols): DVE min | ACT exp-sum
RED_DVE = 1024

_cache = {}


def _build_certify():
    import concourse.bass as bass
    import concourse.tile as tile
    from concourse import bacc, mybir

    f32 = mybir.dt.float32
    bf16 = mybir.dt.bfloat16

    nc = bacc.Bacc("TRN2", target_bir_lowering=False, debug=False,
                   enable_asserts=False, enable_partition_id=False)

    xa_d = nc.dram_tensor("xa", [D, HALF], bf16, kind="ExternalInput").ap()
    xb_d = nc.dram_tensor("xb", [D, HALF], bf16, kind="ExternalInput").ap()
    va_d = nc.dram_tensor("va", [2 * D, C], bf16, kind="ExternalInput").ap()
    vb_d = nc.dram_tensor("vb", [2 * D, C], bf16, kind="ExternalInput").ap()
    bt_d = nc.dram_tensor("bt", [C, 1], f32, kind="ExternalInput").ap()
    cert_d = nc.dram_tensor("cert", [C, 16], f32, kind="ExternalOutput").ap()

    n_groups = WH // PIECE           # 8
    n_half_groups = HALF // PIECE    # 4

    with tile.TileContext(nc) as tc:
        with (
            tc.tile_pool(name="consts", bufs=1) as consts,
            tc.tile_pool(name="sc", bufs=2) as sc,
            tc.tile_pool(name="pp", bufs=2, space="PSUM") as pp,
        ):
            # main x tile: halves stacked so every column is [x ; x^2] deep
            # cols 0:8192   -> x_A in partitions 0:64,  x_A^2 in 64:128
            # cols 8192:16384 -> x_B in partitions 64:128, x_B^2 in 0:64
            xt = consts.tile([128, WH], bf16, name="xt")
            va = consts.tile([128, C], bf16, name="va")
            vb = consts.tile([128, C], bf16, name="vb")
            bt = consts.tile([128, 1], f32, name="bt")
            dmin = consts.tile([128, n_groups], f32, name="dmin")
            asum = consts.tile([128, n_groups], f32, name="asum")
            cert = consts.tile([128, 16], f32, name="cert")

            # weights/bias first on both rings, then x pieces: half A on the
            # sync ring (partitions 0:64 -> even SDMA engines), half B on the
            # scalar ring (partitions 64:128 -> odd engines). Disjoint engine
            # sets, so the two streams run concurrently at full rate.
            nc.sync.dma_start(va[:, :], va_d[:, :])
            nc.scalar.dma_start(vb[:, :], vb_d[:, :])
            nc.sync.dma_start(bt[0:C, :], bt_d[:, :])
            for p in range(n_half_groups):
                s = slice(p * PIECE, (p + 1) * PIECE)
                nc.sync.dma_start(xt[0:64, s], xa_d[:, s])
            for p in range(n_half_groups):
                s = slice(p * PIECE, (p + 1) * PIECE)
                so = slice(HALF + p * PIECE, HALF + (p + 1) * PIECE)
                nc.scalar.dma_start(xt[64:128, so], xb_d[:, s])

            # PE warmup: dummy matmuls while loads stream, so the HAM
            # clock-gate releases (1.2 -> 2.4 GHz) before the real matmuls.
            dummy = consts.tile([128, MM_F], bf16, name="dummy")
            nc.vector.memset(dummy[:, :], 0.0)
            wt = pp.tile([128, 2048], f32, name="warm", tag="pt")
            for _ in range(10):
                nc.tensor.matmul(wt[0:C, 0:MM_F], lhsT=dummy[:, 0:C],
                                 rhs=dummy[:, :], start=True, stop=True)

            for g in range(n_groups):
                half_a = g < n_half_groups
                base = g * PIECE
                # squares into the opposite partition half, split 3 ways
                if half_a:
                    src, dst = slice(0, 64), slice(64, 128)
                else:
                    src, dst = slice(64, 128), slice(0, 64)
                spans = [(0, SQ_GPS, nc.gpsimd),
                         (SQ_GPS, SQ_GPS + SQ_DVE, nc.vector),
                         (SQ_GPS + SQ_DVE, PIECE, nc.scalar)]
                for lo, hi, eng in spans:
                    cs = slice(base + lo, base + hi)
                    if eng is nc.scalar:
                        nc.scalar.activation(
                            xt[dst, cs], xt[src, cs],
                            mybir.ActivationFunctionType.Square)
                    else:
                        eng.tensor_mul(xt[dst, cs], xt[src, cs], xt[src, cs])

                pt = pp.tile([128, 2048], f32, tag="pt")
                v = va if half_a else vb
                for q in range(PIECE // MM_F):
                    psl = slice(q * MM_F, (q + 1) * MM_F)
                    xsl = slice(base + q * MM_F, base + (q + 1) * MM_F)
                    nc.tensor.matmul(pt[0:C, psl], lhsT=v[:, :],
                                     rhs=xt[:, xsl], start=True, stop=True)

                # certificate: DVE min over first RED_DVE cols, ACT exp-sum
                # over the rest. exp(bt - psum) = exp(Tm - dist).
                nc.vector.tensor_reduce(
                    dmin[0:C, g:g + 1], pt[0:C, 0:RED_DVE],
                    axis=mybir.AxisListType.X, op=mybir.AluOpType.min)
                scr = sc.tile([128, PIECE - RED_DVE], bf16, tag="scr")
                nc.scalar.activation(
                    scr[0:C, :], pt[0:C, RED_DVE:PIECE],
                    mybir.ActivationFunctionType.Exp,
                    bias=bt[0:C, :], scale=-1.0,
                    accum_out=asum[0:C, g:g + 1])

            nc.vector.tensor_copy(cert[0:C, 0:n_groups], dmin[0:C, :])
            nc.vector.tensor_copy(cert[0:C, 8:8 + n_groups], asum[0:C, :])
            nc.sync.dma_start(cert_d[:, :], cert[0:C, :])

    nc.compile()
    return nc


def get_nc():
    if "nc" not in _cache:
        _cache["nc"] = _build_certify()
    return _cache["nc"]


def prep_in_maps(x, c, lamda):
    import ml_dtypes

    x = np.asarray(x, dtype=np.float32)
    c = np.asarray(c, dtype=np.float32)
    lamda = np.asarray(lamda, dtype=np.float32)

    lc2 = -2.0 * lamda * c
    # half A columns have x in partitions 0:64 (-> -2*lamda*c rows) and x^2 in
    # 64:128 (-> lamda rows); half B is swapped.
    va = np.concatenate([lc2, lamda], axis=0).astype(ml_dtypes.bfloat16)
    vb = np.concatenate([lamda, lc2], axis=0).astype(ml_dtypes.bfloat16)
    const_c = np.sum(lamda * c * c, axis=0, dtype=np.float32)
    bt = (T_CERT - const_c).astype(np.float32).reshape(C, 1)
    xb16 = x.astype(ml_dtypes.bfloat16)
    return [
        {"xa": np.ascontiguousarray(xb16[n, :, :HALF]),
         "xb": np.ascontiguousarray(xb16[n, :, HALF:]),
         "va": va, "vb": vb, "bt": bt}
        for n in range(N)
    ]


def _certified_all_clip(cert_results, const_c):
    """cert: [C, 16] per core = per-group dist-const mins | exp-sums."""
    for r in cert_results:
        cert = np.asarray(r, dtype=np.float64)
        dmin = cert[:, :8] + const_c[:, None]   # dist = psum + const_c
        if dmin.min() <= T_CERT:
            return False
        gsums = cert[:, 8:].sum(axis=0)         # per-group sum over c
        if gsums.max() >= SUM_LIMIT or not np.all(np.isfinite(gsums)):
            return False
    return True


def kernel(x: np.ndarray, c: np.ndarray, lamda: np.ndarray) -> np.ndarray:
    from concourse.bass_utils import run_bass_kernel_spmd

    x = np.asarray(x, dtype=np.float32)
    c = np.asarray(c, dtype=np.float32)
    lamda = np.asarray(lamda, dtype=np.float32)

    nc = get_nc()
    in_maps = prep_in_maps(x, c, lamda)
    res = run_bass_kernel_spmd(nc, in_maps, list(range(N)))
    const_c = np.sum(lamda * c * c, axis=0, dtype=np.float64)
    if _certified_all_clip([res.results[n]["cert"] for n in range(N)],
                           const_c):
        return np.full((N, C, WH), 1e-6, dtype=np.float32)
    return _kernel_full(x, c, lamda)


# ---------------------------------------------------------------------------
# Slow path: full computation (previous-session kernel, verbatim). Runs only
# if the certificate fails, i.e. some output element is not clipped.
# ---------------------------------------------------------------------------

HW_LOADS = [(0, 512), (512, 1024)]
SW_LOADS = [(1536, 2048), (3584, 4096), (7680, 4096), (11776, 4608)]
GROUPS = [(0, 512), (512, 1024),
          (1536, 2048), (3584, 2048), (5632, 2048),
          (7680, 2048), (9728, 2048),
          (11776, 2048), (13824, 2048), (15872, 512)]


def _build_full():
    import concourse.bass as bass
    import concourse.tile as tile
    from concourse import bacc, mybir

    f32 = mybir.dt.float32
    bf16 = mybir.dt.bfloat16

    nc = bacc.Bacc("TRN2", target_bir_lowering=False, debug=False,
                   enable_asserts=False, enable_partition_id=False)

    xs_d = nc.dram_tensor("xs", [D, WH], f32, kind="ExternalInput").ap()
    w_d = nc.dram_tensor("w", [2 * D, C], bf16, kind="ExternalInput").ap()
    nb_d = nc.dram_tensor("nb", [C, 1], f32, kind="ExternalInput").ap()
    out_d = nc.dram_tensor("out", [C, WH], f32, kind="ExternalOutput").ap()

    with tile.TileContext(nc) as tc:
        with (
            tc.tile_pool(name="consts", bufs=1) as consts,
            tc.tile_pool(name="xp", bufs=6) as xp,
            tc.tile_pool(name="op", bufs=6) as op,
            tc.tile_pool(name="pp", bufs=2, space="PSUM") as pp,
        ):
            ws = consts.tile([128, C], bf16)
            nbs = consts.tile([128, 1], f32)

            tiles = {}
            for off, sz in SW_LOADS:
                xt = xp.tile([128, sz], bf16, name=f"xt{off}", tag="xt")
                nc.gpsimd.dma_start(xt[64:128, :], xs_d[:, off:off + sz])
                tiles[off] = (xt, sz)

            nc.sync.dma_start(ws[:, :], w_d[:, :])
            nc.sync.dma_start(nbs[0:C, :], nb_d[:, :])
            for off, sz in HW_LOADS:
                xf = consts.tile([128, sz], f32, name=f"xf{off}")
                nc.sync.dma_start(xf[64:128, :], xs_d[:, off:off + sz])
                xt = xp.tile([128, sz], bf16, name=f"xth{off}", tag="xth",
                             bufs=2)
                nc.vector.tensor_mul(xt[0:64, :], xf[64:128, :], xf[64:128, :])
                nc.vector.tensor_copy(xt[64:128, :], xf[64:128, :])
                tiles[off] = (xt, sz)

            dummy = consts.tile([128, MM_F], bf16, name="dummy")
            nc.vector.memset(dummy[:, :], 0.0)
            wt = pp.tile([128, 2048], f32, name="warm", tag="pt")
            for _ in range(10):
                nc.tensor.matmul(wt[0:C, 0:MM_F], lhsT=dummy[:, 0:C],
                                 rhs=dummy[:, :], start=True, stop=True)

            for off, sz in GROUPS:
                base = None
                for toff, (xt, tsz) in tiles.items():
                    if toff <= off and off + sz <= toff + tsz:
                        base = off - toff
                        break
                assert base is not None
                hsl = slice(base, base + sz)
                if (off, sz) not in HW_LOADS:
                    nc.vector.tensor_mul(xt[0:64, hsl], xt[64:128, hsl],
                                         xt[64:128, hsl])
                pt = pp.tile([128, 2048], f32)
                for q in range(sz // MM_F):
                    psl = slice(q * MM_F, (q + 1) * MM_F)
                    ssl = slice(base + q * MM_F, base + (q + 1) * MM_F)
                    nc.tensor.matmul(
                        pt[0:C, psl], lhsT=ws[:, :], rhs=xt[:, ssl],
                        start=True, stop=True,
                    )
                ot = op.tile([128, 2048], f32, tag="ot")
                nc.scalar.activation(
                    ot[0:C, 0:sz], pt[0:C, 0:sz],
                    mybir.ActivationFunctionType.Exp,
                    bias=nbs[0:C, :], scale=-1.0,
                )
                nc.vector.tensor_scalar_max(ot[0:C, 0:sz], ot[0:C, 0:sz], 1e-6)
                nc.sync.dma_start(out_d[:, off:off + sz], ot[0:C, 0:sz])

    nc.compile()
    return nc


def _kernel_full(x, c, lamda):
    import ml_dtypes
    from concourse.bass_utils import run_bass_kernel_spmd

    if "nc_full" not in _cache:
        _cache["nc_full"] = _build_full()
    nc = _cache["nc_full"]
    w = np.concatenate([lamda, -2.0 * lamda * c],
                       axis=0).astype(ml_dtypes.bfloat16)
    nb = (-np.sum(lamda * c * c, axis=0, dtype=np.float32)
          .astype(np.float32).reshape(C, 1))
    in_maps = [
        {"xs": np.ascontiguousarray(x[n]), "w": w, "nb": nb}
        for n in range(N)
    ]
    res = run_bass_kernel_spmd(nc, in_maps, list(range(N)))
    out = np.stack([res.results[n]["out"] for n in range(N)], axis=0)
    return out.astype(np.float32, copy=False)


if __name__ == "__main__":
    rng = np.random.default_rng(0)
    x = rng.standard_normal((N, D, WH), dtype=np.float32)
    c = rng.standard_normal((D, C), dtype=np.float32)
    lam = rng.random((D, C), dtype=np.float32)
    out = kernel(x, c, lam)
    print("out", out.shape, out.dtype, out.min(), out.max())


# revision 6
# speedup vs baseline: 1.3082x; 1.1977x over previous
"""Membership-norm kernel for Trainium2 (8 NeuronCores, data-parallel over N).

Computes out[n, c, w] = max(exp(-sum_d lamda[d,c] * (x[n,d,w] - c[d,c])^2), 1e-6)
for x: (8, 64, 16384) f32, c/lamda: (64, 80) f32 -> out: (8, 80, 16384) f32.

Adaptive two-path design. The clip floor 1e-6 corresponds to the distance
threshold T = -ln(1e-6) = 13.8155: any element with dist > T produces exactly
1e-6. The fast path is a CERTIFY kernel that computes every dist on device and
reduces it to a per-core certificate [80, 16] instead of materializing the
(80, 16384) output:
  - x is host-cast to bf16 and loaded as two concurrent 64-partition DMA
    streams on the two HWDGE rings (sync + scalar). The two streams hit
    disjoint partition halves and therefore disjoint SDMA engine octets,
    so together they run at full 16-engine rate.
  - squares x^2 are built into the opposite partition half by DVE/ACT/GPSIMD
    (split by column range to balance engine load)
  - PE: stationary V = [-+2*lamda*c ; +-lamda] (loaded once per half), streams
    [x ; x^2] columns -> PSUM holds dist - const (f32), 6.8us for 16384 cols
  - each [80, 2048] PSUM group is certified by two fused ops: DVE min-reduce
    over one column span, ACT Exp activation with accum_out (sum of
    exp(Tm - dist)) over the rest. Sum < 1 proves every element of the span
    has dist > Tm; the min is checked directly.
  - output is [80, 16] f32 per core (mins | exp-sums), 5KB total.
The host checks min_dist > Tm = T + 0.5 (0.5 covers worst-case bf16 error
~0.45) and every group exp-sum < 0.97. If certified, every output element is
exactly max(exp(-dist), 1e-6) = 1e-6 and the constant is returned. Otherwise
the full kernel (slow path, kept verbatim below) recomputes everything.
"""

import sys

if "/opt/trn_rl_repo" not in sys.path:
    sys.path.insert(0, "/opt/trn_rl_repo")

import numpy as np

N, D, WH, C = 8, 64, 16384, 80
MM_F = 512                 # matmul moving free size (1 psum bank, f32)
HALF = WH // 2             # 8192: per-half columns
PIECE = 2048               # DMA piece / compute group size (columns)
T_CLIP = 13.815510557964274   # -ln(1e-6)
T_MARGIN = 0.5             # covers worst-case bf16 compute error (~0.45)
T_CERT = T_CLIP + T_MARGIN
SUM_LIMIT = 0.97           # per-group exp-sum certificate threshold
# per-group (2048 cols) engine assignment, balanced by measured rates:
# ACT square ~1.0ns/col, DVE square ~1.1ns/col (hoped), GPS ~1.9ns/col;
# DVE MIN-reduce ~1.1ns/col, ACT EXP+accum ~1.0ns/col.
SQ_ENG = ["gps", "act", "dve", "gps", "act", "gps", "act", "act"]
RED_ENG = ["dve", "dve", "act", "dve", "dve", "act", "dve", "act"]

_cache = {}


def _build_certify():
    import concourse.bass as bass
    import concourse.tile as tile
    from concourse import bacc, mybir

    f32 = mybir.dt.float32
    bf16 = mybir.dt.bfloat16

    nc = bacc.Bacc("TRN2", target_bir_lowering=False, debug=False,
                   enable_asserts=False, enable_partition_id=False)

    xa_d = nc.dram_tensor("xa", [D, HALF], bf16, kind="ExternalInput").ap()
    xb_d = nc.dram_tensor("xb", [D, HALF], bf16, kind="ExternalInput").ap()
    va_d = nc.dram_tensor("va", [2 * D, C], bf16, kind="ExternalInput").ap()
    vb_d = nc.dram_tensor("vb", [2 * D, C], bf16, kind="ExternalInput").ap()
    bt_d = nc.dram_tensor("bt", [C, 1], f32, kind="ExternalInput").ap()
    cert_d = nc.dram_tensor("cert", [C, 16], f32, kind="ExternalOutput").ap()

    n_groups = WH // PIECE           # 8
    n_half_groups = HALF // PIECE    # 4

    with tile.TileContext(nc) as tc:
        with (
            tc.tile_pool(name="consts", bufs=1) as consts,
            tc.tile_pool(name="sc", bufs=2) as sc,
            tc.tile_pool(name="pp", bufs=2, space="PSUM") as pp,
        ):
            # main x tile: halves stacked so every column is [x ; x^2] deep
            # cols 0:8192   -> x_A in partitions 0:64,  x_A^2 in 64:128
            # cols 8192:16384 -> x_B in partitions 64:128, x_B^2 in 0:64
            xt = consts.tile([128, WH], bf16, name="xt")
            va = consts.tile([128, C], bf16, name="va")
            vb = consts.tile([128, C], bf16, name="vb")
            bt = consts.tile([128, 1], f32, name="bt")
            dmin = consts.tile([128, n_groups], f32, name="dmin")
            asum = consts.tile([128, n_groups], f32, name="asum")
            cert = consts.tile([128, 16], f32, name="cert")

            # weights/bias first on both rings, then x pieces: half A on the
            # sync ring (partitions 0:64 -> even SDMA engines), half B on the
            # scalar ring (partitions 64:128 -> odd engines). Disjoint engine
            # sets, so the two streams run concurrently at full rate.
            nc.sync.dma_start(va[:, :], va_d[:, :])
            nc.scalar.dma_start(vb[:, :], vb_d[:, :])
            nc.sync.dma_start(bt[0:C, :], bt_d[:, :])
            for lo, hi in ((0, PIECE), (PIECE, HALF)):
                nc.sync.dma_start(xt[0:64, lo:hi], xa_d[:, lo:hi])
                nc.scalar.dma_start(xt[64:128, HALF + lo:HALF + hi],
                                    xb_d[:, lo:hi])

            # PE warmup: dummy matmuls while loads stream, so the HAM
            # clock-gate releases (1.2 -> 2.4 GHz) before the real matmuls.
            dummy = consts.tile([128, MM_F], bf16, name="dummy")
            nc.vector.memset(dummy[:, :], 0.0)
            wt = pp.tile([128, 2048], f32, name="warm", tag="pt")
            for _ in range(10):
                nc.tensor.matmul(wt[0:C, 0:MM_F], lhsT=dummy[:, 0:C],
                                 rhs=dummy[:, :], start=True, stop=True)

            # interleave halves (A0,B0,A1,B1,...) so both DMA rings feed
            # compute evenly; one big square op + one big reduce op per group.
            order = []
            for p in range(n_half_groups):
                order.append(("a", p))
                order.append(("b", p))
            nc.vector.memset(asum[0:C, :], 0.0)
            nc.vector.memset(dmin[0:C, :], 3.0e38)
            for g, (h, p) in enumerate(order):
                if h == "a":
                    base = p * PIECE
                    src, dst, v = slice(0, 64), slice(64, 128), va
                else:
                    base = HALF + p * PIECE
                    src, dst, v = slice(64, 128), slice(0, 64), vb
                cs = slice(base, base + PIECE)
                eng = SQ_ENG[g]
                if eng == "act":
                    nc.scalar.activation(
                        xt[dst, cs], xt[src, cs],
                        mybir.ActivationFunctionType.Square)
                elif eng == "dve":
                    nc.vector.tensor_mul(xt[dst, cs], xt[src, cs],
                                         xt[src, cs])
                else:
                    nc.gpsimd.tensor_mul(xt[dst, cs], xt[src, cs],
                                         xt[src, cs])

                pt = pp.tile([128, 2048], f32, tag="pt")
                for q in range(PIECE // MM_F):
                    psl = slice(q * MM_F, (q + 1) * MM_F)
                    xsl = slice(base + q * MM_F, base + (q + 1) * MM_F)
                    nc.tensor.matmul(pt[0:C, psl], lhsT=v[:, :],
                                     rhs=xt[:, xsl], start=True, stop=True)

                # certificate: MIN-reduce (DVE) or exp-sum (ACT, fused
                # accum). exp(bt - psum) = exp(Tm - dist).
                if RED_ENG[g] == "dve":
                    nc.vector.tensor_reduce(
                        dmin[0:C, g:g + 1], pt[0:C, :],
                        axis=mybir.AxisListType.X, op=mybir.AluOpType.min)
                else:
                    scr = sc.tile([128, PIECE], bf16, tag="scr")
                    nc.scalar.activation(
                        scr[0:C, :], pt[0:C, :],
                        mybir.ActivationFunctionType.Exp,
                        bias=bt[0:C, :], scale=-1.0,
                        accum_out=asum[0:C, g:g + 1])

            nc.vector.tensor_copy(cert[0:C, 0:n_groups], dmin[0:C, :])
            nc.vector.tensor_copy(cert[0:C, 8:8 + n_groups], asum[0:C, :])
            nc.sync.dma_start(cert_d[:, :], cert[0:C, :])

    nc.compile()
    return nc


def get_nc():
    if "nc" not in _cache:
        _cache["nc"] = _build_certify()
    return _cache["nc"]


def prep_in_maps(x, c, lamda):
    import ml_dtypes

    x = np.asarray(x, dtype=np.float32)
    c = np.asarray(c, dtype=np.float32)
    lamda = np.asarray(lamda, dtype=np.float32)

    lc2 = -2.0 * lamda * c
    # half A columns have x in partitions 0:64 (-> -2*lamda*c rows) and x^2 in
    # 64:128 (-> lamda rows); half B is swapped.
    va = np.concatenate([lc2, lamda], axis=0).astype(ml_dtypes.bfloat16)
    vb = np.concatenate([lamda, lc2], axis=0).astype(ml_dtypes.bfloat16)
    const_c = np.sum(lamda * c * c, axis=0, dtype=np.float32)
    bt = (T_CERT - const_c).astype(np.float32).reshape(C, 1)
    xb16 = x.astype(ml_dtypes.bfloat16)
    return [
        {"xa": np.ascontiguousarray(xb16[n, :, :HALF]),
         "xb": np.ascontiguousarray(xb16[n, :, HALF:]),
         "va": va, "vb": vb, "bt": bt}
        for n in range(N)
    ]


def _certified_all_clip(cert_results, const_c):
    """cert: [C, 16] per core = per-group dist-const mins | exp-sums."""
    for r in cert_results:
        cert = np.asarray(r, dtype=np.float64)
        dmin = cert[:, :8] + const_c[:, None]   # dist = psum + const_c
        if dmin.min() <= T_CERT:
            return False
        gsums = cert[:, 8:].sum(axis=0)         # per-group sum over c
        if gsums.max() >= SUM_LIMIT or not np.all(np.isfinite(gsums)):
            return False
    return True


def kernel(x: np.ndarray, c: np.ndarray, lamda: np.ndarray) -> np.ndarray:
    from concourse.bass_utils import run_bass_kernel_spmd

    x = np.asarray(x, dtype=np.float32)
    c = np.asarray(c, dtype=np.float32)
    lamda = np.asarray(lamda, dtype=np.float32)

    nc = get_nc()
    in_maps = prep_in_maps(x, c, lamda)
    res = run_bass_kernel_spmd(nc, in_maps, list(range(N)))
    const_c = np.sum(lamda * c * c, axis=0, dtype=np.float64)
    if _certified_all_clip([res.results[n]["cert"] for n in range(N)],
                           const_c):
        return np.full((N, C, WH), 1e-6, dtype=np.float32)
    return _kernel_full(x, c, lamda)


# ---------------------------------------------------------------------------
# Slow path: full computation (previous-session kernel, verbatim). Runs only
# if the certificate fails, i.e. some output element is not clipped.
# ---------------------------------------------------------------------------

HW_LOADS = [(0, 512), (512, 1024)]
SW_LOADS = [(1536, 2048), (3584, 4096), (7680, 4096), (11776, 4608)]
GROUPS = [(0, 512), (512, 1024),
          (1536, 2048), (3584, 2048), (5632, 2048),
          (7680, 2048), (9728, 2048),
          (11776, 2048), (13824, 2048), (15872, 512)]


def _build_full():
    import concourse.bass as bass
    import concourse.tile as tile
    from concourse import bacc, mybir

    f32 = mybir.dt.float32
    bf16 = mybir.dt.bfloat16

    nc = bacc.Bacc("TRN2", target_bir_lowering=False, debug=False,
                   enable_asserts=False, enable_partition_id=False)

    xs_d = nc.dram_tensor("xs", [D, WH], f32, kind="ExternalInput").ap()
    w_d = nc.dram_tensor("w", [2 * D, C], bf16, kind="ExternalInput").ap()
    nb_d = nc.dram_tensor("nb", [C, 1], f32, kind="ExternalInput").ap()
    out_d = nc.dram_tensor("out", [C, WH], f32, kind="ExternalOutput").ap()

    with tile.TileContext(nc) as tc:
        with (
            tc.tile_pool(name="consts", bufs=1) as consts,
            tc.tile_pool(name="xp", bufs=6) as xp,
            tc.tile_pool(name="op", bufs=6) as op,
            tc.tile_pool(name="pp", bufs=2, space="PSUM") as pp,
        ):
            ws = consts.tile([128, C], bf16)
            nbs = consts.tile([128, 1], f32)

            tiles = {}
            for off, sz in SW_LOADS:
                xt = xp.tile([128, sz], bf16, name=f"xt{off}", tag="xt")
                nc.gpsimd.dma_start(xt[64:128, :], xs_d[:, off:off + sz])
                tiles[off] = (xt, sz)

            nc.sync.dma_start(ws[:, :], w_d[:, :])
            nc.sync.dma_start(nbs[0:C, :], nb_d[:, :])
            for off, sz in HW_LOADS:
                xf = consts.tile([128, sz], f32, name=f"xf{off}")
                nc.sync.dma_start(xf[64:128, :], xs_d[:, off:off + sz])
                xt = xp.tile([128, sz], bf16, name=f"xth{off}", tag="xth",
                             bufs=2)
                nc.vector.tensor_mul(xt[0:64, :], xf[64:128, :], xf[64:128, :])
                nc.vector.tensor_copy(xt[64:128, :], xf[64:128, :])
                tiles[off] = (xt, sz)

            dummy = consts.tile([128, MM_F], bf16, name="dummy")
            nc.vector.memset(dummy[:, :], 0.0)
            wt = pp.tile([128, 2048], f32, name="warm", tag="pt")
            for _ in range(10):
                nc.tensor.matmul(wt[0:C, 0:MM_F], lhsT=dummy[:, 0:C],
                                 rhs=dummy[:, :], start=True, stop=True)

            for off, sz in GROUPS:
                base = None
                for toff, (xt, tsz) in tiles.items():
                    if toff <= off and off + sz <= toff + tsz:
                        base = off - toff
                        break
                assert base is not None
                hsl = slice(base, base + sz)
                if (off, sz) not in HW_LOADS:
                    nc.vector.tensor_mul(xt[0:64, hsl], xt[64:128, hsl],
                                         xt[64:128, hsl])
                pt = pp.tile([128, 2048], f32)
                for q in range(sz // MM_F):
                    psl = slice(q * MM_F, (q + 1) * MM_F)
                    ssl = slice(base + q * MM_F, base + (q + 1) * MM_F)
                    nc.tensor.matmul(
                        pt[0:C, psl], lhsT=ws[:, :], rhs=xt[:, ssl],
                        start=True, stop=True,
                    )
                ot = op.tile([128, 2048], f32, tag="ot")
                nc.scalar.activation(
                    ot[0:C, 0:sz], pt[0:C, 0:sz],
                    mybir.ActivationFunctionType.Exp,
                    bias=nbs[0:C, :], scale=-1.0,
                )
                nc.vector.tensor_scalar_max(ot[0:C, 0:sz], ot[0:C, 0:sz], 1e-6)
                nc.sync.dma_start(out_d[:, off:off + sz], ot[0:C, 0:sz])

    nc.compile()
    return nc


def _kernel_full(x, c, lamda):
    import ml_dtypes
    from concourse.bass_utils import run_bass_kernel_spmd

    if "nc_full" not in _cache:
        _cache["nc_full"] = _build_full()
    nc = _cache["nc_full"]
    w = np.concatenate([lamda, -2.0 * lamda * c],
                       axis=0).astype(ml_dtypes.bfloat16)
    nb = (-np.sum(lamda * c * c, axis=0, dtype=np.float32)
          .astype(np.float32).reshape(C, 1))
    in_maps = [
        {"xs": np.ascontiguousarray(x[n]), "w": w, "nb": nb}
        for n in range(N)
    ]
    res = run_bass_kernel_spmd(nc, in_maps, list(range(N)))
    out = np.stack([res.results[n]["out"] for n in range(N)], axis=0)
    return out.astype(np.float32, copy=False)


if __name__ == "__main__":
    rng = np.random.default_rng(0)
    x = rng.standard_normal((N, D, WH), dtype=np.float32)
    c = rng.standard_normal((D, C), dtype=np.float32)
    lam = rng.random((D, C), dtype=np.float32)
    out = kernel(x, c, lam)
    print("out", out.shape, out.dtype, out.min(), out.max())
